# revision 9
# baseline (speedup 1.0000x reference)
"""Trainium2 Bass kernel for nn_KANPointNet.

Structural insight: every KAN layer wires output channel j to input channel
j % Cin.  Walking the graph backward from the 40 output channels, only
channels 0..39 of layers 1, 2, 6, 7, 8, 9, 10 are live, and layer 6 reads
concat channels 0..39 which all fall in the `local` (layer-2) part — so the
entire max-pool branch (layers 3, 4, 5 + global pooling) is dead code.  The
network reduces to 40 independent per-channel chains of 7 cubic-spline
evaluations (+ ReLU between layers).

Numerical contract: the splines are DISCONTINUOUS at the knots and
intermediate values pass within 1 ulp of knot boundaries, so interval
selection must match the reference bit-for-bit.  XLA-CPU evaluates the
Horner polynomial with separately-rounded mult/add (verified: no FMA
contraction), which the per-op-rounded vector-engine ALUs reproduce
exactly.  Coefficient/knot selection uses one-hot masks (products with
exact 0.0/1.0), which is exact in any rounding mode.

Distribution: pure data-parallel over the B*N = 65536 points, 8192 per
core; no collectives (the max-pool that would have needed an
all-reduce-max is dead).  On-chip layout packs 3 point-groups x 40
channels onto 120 partitions; per-channel spline coefficients ride along
as per-partition scalars.
"""

import numpy as np

NCORES = 8
B, CIN, N = 8, 3, 8192
CH = 40                      # live channels
LAYERS = (1, 2, 6, 7, 8, 9, 10)
NL = len(LAYERS)
NI = 5                       # spline intervals (K-1)
GROUPS = 3
P = GROUPS * CH              # 120 partitions
PTS = B * N                  # 65536 total points
PTS_CORE = PTS // NCORES     # 8192
FREE = -(-PTS_CORE // GROUPS)  # 2731 (one padded point per core)
CPL = 24                     # const columns per layer: 20 coefs + 4 knots
NCHUNK = 3

_prog_cache = {}


def _build_program():
    """Build the Bass/Tile program (shape-only; all values arrive via DRAM)."""
    import concourse.bass as bass  # noqa: F401
    import concourse.mybir as mybir
    from concourse import bacc, tile

    f32 = mybir.dt.float32
    Alu = mybir.AluOpType
    Act = mybir.ActivationFunctionType

    nc = bacc.Bacc(None, target_bir_lowering=False, debug=True)
    v0_d = nc.declare_dram_parameter("v0", [P, FREE], f32, isOutput=False)
    c_d = nc.declare_dram_parameter("consts", [P, NL * CPL], f32, isOutput=False)
    o_d = nc.declare_dram_parameter("out", [P, FREE], f32, isOutput=True)

    fsz = [FREE // NCHUNK + (1 if i < FREE % NCHUNK else 0) for i in range(NCHUNK)]
    foff = [sum(fsz[:i]) for i in range(NCHUNK)]

    with tile.TileContext(nc) as tc:
        with (
            tc.tile_pool(name="cpool", bufs=1) as cpool,
            tc.tile_pool(name="vpool", bufs=2 * NCHUNK) as vpool,
            tc.tile_pool(name="pool", bufs=2) as pool,
        ):
            consts = cpool.tile([P, NL * CPL], f32, tag="consts")
            nc.sync.dma_start(consts[:], c_d[:])

            vcur = []
            for u in range(NCHUNK):
                vt = vpool.tile([P, fsz[u]], f32, tag="v")
                nc.sync.dma_start(vt[:], v0_d[:, foff[u]:foff[u] + fsz[u]])
                vcur.append(vt)

            for li in range(NL):
                cb = li * CPL

                def cc(m, k):
                    # coef k of interval m, per-partition scalar column
                    return consts[:, cb + m * 4 + k:cb + m * 4 + k + 1]

                def kt(j):
                    # knot t_{j+1} (j = 0..3)
                    return consts[:, cb + 20 + j:cb + 20 + j + 1]

                vnext = []
                for u in range(NCHUNK):
                    F = fsz[u]
                    V = vcur[u]
                    s = []
                    for j in range(4):
                        st = pool.tile([P, F], f32, tag=f"mask{j}")
                        nc.vector.tensor_scalar(
                            out=st[:], in0=V[:], scalar1=kt(j), scalar2=None,
                            op0=Alu.is_ge)
                        s.append(st)
                    ind0 = pool.tile([P, F], f32, tag="ind0")
                    nc.vector.tensor_scalar(
                        out=ind0[:], in0=s[0][:], scalar1=-1.0, scalar2=1.0,
                        op0=Alu.mult, op1=Alu.add)
                    ind = [ind0]
                    for j in range(3):
                        it = pool.tile([P, F], f32, tag=f"ind{j + 1}")
                        nc.gpsimd.tensor_tensor(
                            out=it[:], in0=s[j][:], in1=s[j + 1][:],
                            op=Alu.subtract)
                        ind.append(it)
                    ind.append(s[3])  # ind4 == s4

                    # knot select: T = sum_m ind_m * t_m   (t_0 == 0 skipped)
                    T = pool.tile([P, F], f32, tag="tsel")
                    nc.vector.tensor_scalar(
                        out=T[:], in0=ind[1][:], scalar1=kt(0), scalar2=None,
                        op0=Alu.mult)
                    for m in (2, 3, 4):
                        nc.vector.scalar_tensor_tensor(
                            out=T[:], in0=ind[m][:], scalar=kt(m - 1),
                            in1=T[:], op0=Alu.mult, op1=Alu.add)
                    dx = pool.tile([P, F], f32, tag="dx")
                    nc.vector.tensor_tensor(
                        out=dx[:], in0=V[:], in1=T[:], op=Alu.subtract)

                    # one-hot coefficient selection; c2/c3 chains on gpsimd
                    X = []
                    for k in range(4):
                        eng = nc.vector
                        xt = pool.tile([P, F], f32, tag=f"x{k}")
                        eng.tensor_scalar(
                            out=xt[:], in0=ind[0][:], scalar1=cc(0, k),
                            scalar2=None, op0=Alu.mult)
                        for m in range(1, 5):
                            eng.scalar_tensor_tensor(
                                out=xt[:], in0=ind[m][:], scalar=cc(m, k),
                                in1=xt[:], op0=Alu.mult, op1=Alu.add)
                        X.append(xt)

                    # Horner, separately-rounded to match the reference:
                    # y = ((c0*dx + c1)*dx + c2)*dx + c3
                    h = pool.tile([P, F], f32, tag="h")
                    y = pool.tile([P, F], f32, tag="y")
                    nc.vector.tensor_tensor(out=h[:], in0=X[0][:], in1=dx[:], op=Alu.mult)
                    nc.vector.tensor_tensor(out=h[:], in0=h[:], in1=X[1][:], op=Alu.add)
                    nc.vector.tensor_tensor(out=h[:], in0=h[:], in1=dx[:], op=Alu.mult)
                    nc.vector.tensor_tensor(out=h[:], in0=h[:], in1=X[2][:], op=Alu.add)
                    nc.vector.tensor_tensor(out=h[:], in0=h[:], in1=dx[:], op=Alu.mult)
                    nc.vector.tensor_tensor(out=y[:], in0=h[:], in1=X[3][:], op=Alu.add)

                    if li < NL - 1:
                        vn = vpool.tile([P, F], f32, tag="v")
                        nc.scalar.activation(out=vn[:], in_=y[:], func=Act.Relu)
                        vnext.append(vn)
                    else:
                        nc.sync.dma_start(
                            o_d[:, foff[u]:foff[u] + fsz[u]], y[:])
                vcur = vnext

    nc.compile()
    return nc


def _get_program():
    if "nc" not in _prog_cache:
        _prog_cache["nc"] = _build_program()
    return _prog_cache["nc"]


def _pack_inputs(inputs):
    x = np.ascontiguousarray(np.asarray(inputs["x"], dtype=np.float32))
    assert x.shape == (B, CIN, N), x.shape

    # consts: identical for every core
    consts = np.zeros((P, NL * CPL), dtype=np.float32)
    for li, ref_l in enumerate(LAYERS):
        kn = np.asarray(inputs[f"knots{ref_l}"], dtype=np.float32)[:CH]
        cf = np.asarray(inputs[f"coefs{ref_l}"], dtype=np.float32)[:CH]
        assert np.all(kn[:, 0] == 0.0), "kernel assumes knots start at 0"
        assert np.all(kn == kn[0][None, :]), "kernel assumes shared knots per layer"
        base = li * CPL
        for m in range(NI):
            for k in range(4):
                consts[:, base + m * 4 + k] = np.tile(cf[:, k, m], GROUPS)
        for j in range(4):
            consts[:, base + 20 + j] = kn[0, j + 1]

    # v0: expand x to the 40 live channels (channel j reads x[j % 3]),
    # shard points across cores, pad to GROUPS*FREE, pack [P, FREE]
    xf = x.transpose(1, 0, 2).reshape(CIN, PTS)          # [3, 65536]
    v0 = xf[np.arange(CH) % CIN]                         # [40, 65536]
    shards = []
    for c in range(NCORES):
        vc = v0[:, c * PTS_CORE:(c + 1) * PTS_CORE]      # [40, 8192]
        pad = GROUPS * FREE - PTS_CORE
        if pad:
            vc = np.concatenate(
                [vc, np.full((CH, pad), 0.5, dtype=np.float32)], axis=1)
        vc = vc.reshape(CH, GROUPS, FREE).transpose(1, 0, 2).reshape(P, FREE)
        shards.append(np.ascontiguousarray(vc))
    return shards, consts


def _unpack_outputs(outs):
    # outs: list of [P, FREE] per core -> [B, CH, N]
    cols = []
    for c in range(NCORES):
        yc = np.asarray(outs[c]).reshape(GROUPS, CH, FREE)
        yc = yc.transpose(1, 0, 2).reshape(CH, GROUPS * FREE)[:, :PTS_CORE]
        cols.append(yc)
    y = np.concatenate(cols, axis=1)                     # [40, 65536]
    return np.ascontiguousarray(
        y.reshape(CH, B, N).transpose(1, 0, 2)).astype(np.float32)


def run(inputs, trace=False):
    """Run on the 8 NeuronCores; returns (output, BassKernelResults)."""
    from concourse.bass_utils import run_bass_kernel_spmd
    from concourse.bass_interp import get_hw_module

    shards, consts = _pack_inputs(inputs)
    nc = _get_program()
    in_maps = [{"v0": shards[c], "consts": consts} for c in range(NCORES)]
    old_m = nc.m
    nc.m = get_hw_module(nc.m)
    try:
        res = run_bass_kernel_spmd(
            nc, in_maps, core_ids=list(range(NCORES)), trace=trace)
    finally:
        nc.m = old_m
    out = _unpack_outputs([r["out"] for r in res.results])
    return out, res


def kernel(**inputs) -> np.ndarray:
    out, _ = run(inputs, trace=False)
    return out
